# revision 3
# baseline (speedup 1.0000x reference)
"""Trainium2 Bass kernel for AssignClsLabel (clipped-IoU >= 0.7 proposal labeling).

Problem: bboxess [8, 65536, 4] f32, gt_bboxess [8, 64, 4] f32,
gt_counts/counts [8,1] int. Output labels [8, 65536, 1] int (0/1).

Only proposals n < count_b and gts a < gt_count_b matter (labels elsewhere
are 0), which is ~16% of the full N*A pair grid for this dataset. The
kernel therefore packs WORK UNITS = (batch b, chunk of Q proposals,
group of G gts) across 8 cores x 128 partitions x T iterations; every
partition-slot carries its own per-gt scalar columns, so different
partitions process different batches simultaneously.

Device math per pair (all f32; validated 0 label flips vs reference on
the fixed dataset):
    r2(v) = relu(d21 - relu(v - g1))       [= g2 - clip(v, g1, g2)]
    dy = r2(y1) - r2(y2)   (= clip(y2) - clip(y1));  dx likewise
    i  = dy * dx
    c  = area + ga;  fire <=> (0.41c - i)(c - i) <= 0
                     <=> |i - (12/17) c| - (5/17)|c| <= 0
Engine split: ACT does the 8 relu clips + c-terms (Identity/Abs with
per-partition bias columns, ~0.54 ns/elem packed); DVE does the 6
two-tensor ops (1.08 ns/elem). Both engines land ~balanced.
Invalid/duplicate gt slots are filled with real gts (min over
duplicates is harmless) or a far box that can never fire.
"""
import sys

import numpy as np

if "/opt/trn_rl_repo" not in sys.path:
    sys.path.insert(0, "/opt/trn_rl_repo")

import concourse.mybir as mybir
import concourse.tile as tile
from concourse import bacc
from concourse.bass_utils import run_bass_kernel_spmd

AOP = mybir.AluOpType
ACT = mybir.ActivationFunctionType
F32 = mybir.dt.float32

P = 128          # SBUF partitions
Q = 512          # proposals per work unit
G = 4            # gts per work unit
N_CORES = 8
C1217 = np.float32(12.0 / 17.0)
C517 = np.float32(5.0 / 17.0)

# scal columns per slot, per gt j in 0..G-1
S_NGY1 = 0       # -gy1
S_D21Y = G       # gy2 - gy1
S_NGX1 = 2 * G   # -gx1
S_E21X = 3 * G   # gx2 - gx1
S_GA1217 = 4 * G  # (12/17) ga
S_GA517 = 5 * G   # (5/17) ga
SCAL_W = 6 * G

FQ = 6 * Q       # feature width per slot: y1,y2,x1,x2,area1217,area517


def make_plan(inputs):
    counts = inputs["counts"]
    gt_counts = inputs["gt_counts"]
    B = counts.shape[0]
    units = []   # (b, n0, L, gt_idx tuple)
    for b in range(B):
        cnt = int(counts[b, 0])
        gcnt = int(gt_counts[b, 0])
        if cnt <= 0 or gcnt <= 0:
            continue
        nchunks = -(-cnt // Q) if cnt >= Q else 1
        ngroups = -(-gcnt // G)
        for k in range(nchunks):
            n0 = min(k * Q, max(0, cnt - Q))
            L = min(Q, cnt - n0)
            for g in range(ngroups):
                a0 = min(g * G, max(0, gcnt - G))
                gt_idx = tuple(min(a0 + j, gcnt - 1) for j in range(G))
                units.append((b, n0, L, gt_idx))
    T = -(-len(units) // (N_CORES * P))
    return {"units": units, "T": T}


def build_graph(plan):
    T = plan["T"]
    nc = bacc.Bacc()
    feat_d = nc.declare_dram_parameter("feat", [P, T * FQ], F32, isOutput=False)
    scal_d = nc.declare_dram_parameter("scal", [P, T * SCAL_W], F32,
                                       isOutput=False)
    out_d = nc.declare_dram_parameter("out", [P, T * Q], F32, isOutput=True)

    with tile.TileContext(nc) as tc:
        with (
            tc.tile_pool(name="ft", bufs=2) as fp,
            tc.tile_pool(name="gt", bufs=2) as gp,
            tc.tile_pool(name="wide", bufs=2) as wp,
        ):
            for t in range(T):
                ftile = fp.tile([P, FQ], F32, tag="feat", name=f"feat{t}")
                stile = fp.tile([P, SCAL_W], F32, tag="scal", name=f"scal{t}")
                nc.sync.dma_start(ftile[:], feat_d[:, t * FQ:(t + 1) * FQ])
                nc.sync.dma_start(stile[:], scal_d[:,
                                                   t * SCAL_W:(t + 1) * SCAL_W])
                fy1 = ftile[:, 0 * Q:1 * Q]
                fy2 = ftile[:, 1 * Q:2 * Q]
                fx1 = ftile[:, 2 * Q:3 * Q]
                fx2 = ftile[:, 3 * Q:4 * Q]
                fa1217 = ftile[:, 4 * Q:5 * Q]
                fa517 = ftile[:, 5 * Q:6 * Q]

                def col(base, j, stile=stile):
                    return stile[:, base + j:base + j + 1]

                dyt = wp.tile([P, G * Q], F32, tag="dyt", name=f"dyt{t}")
                dxt = wp.tile([P, G * Q], F32, tag="dxt", name=f"dxt{t}")
                c07 = wp.tile([P, G * Q], F32, tag="c07", name=f"c07{t}")
                cab = wp.tile([P, G * Q], F32, tag="cab", name=f"cab{t}")
                for j in range(G):
                    s = slice(j * Q, (j + 1) * Q)
                    r1a = gp.tile([P, Q], F32, tag="r1a", name=f"r1a{t}_{j}")
                    r1b = gp.tile([P, Q], F32, tag="r1b", name=f"r1b{t}_{j}")
                    r2y1 = gp.tile([P, Q], F32, tag="r2y1", name=f"r2y1{t}_{j}")
                    r2y2 = gp.tile([P, Q], F32, tag="r2y2", name=f"r2y2{t}_{j}")
                    r2x1 = gp.tile([P, Q], F32, tag="r2x1", name=f"r2x1{t}_{j}")
                    r2x2 = gp.tile([P, Q], F32, tag="r2x2", name=f"r2x2{t}_{j}")
                    nc.scalar.activation(r1a[:], fy1, ACT.Relu,
                                         bias=col(S_NGY1, j))
                    nc.scalar.activation(r2y1[:], r1a[:], ACT.Relu,
                                         bias=col(S_D21Y, j), scale=-1.0)
                    nc.scalar.activation(r1b[:], fy2, ACT.Relu,
                                         bias=col(S_NGY1, j))
                    nc.scalar.activation(r2y2[:], r1b[:], ACT.Relu,
                                         bias=col(S_D21Y, j), scale=-1.0)
                    nc.vector.tensor_tensor(dyt[:, s], r2y1[:], r2y2[:],
                                            AOP.subtract)
                    nc.scalar.activation(r1a[:], fx1, ACT.Relu,
                                         bias=col(S_NGX1, j))
                    nc.scalar.activation(r2x1[:], r1a[:], ACT.Relu,
                                         bias=col(S_E21X, j), scale=-1.0)
                    nc.scalar.activation(r1b[:], fx2, ACT.Relu,
                                         bias=col(S_NGX1, j))
                    nc.scalar.activation(r2x2[:], r1b[:], ACT.Relu,
                                         bias=col(S_E21X, j), scale=-1.0)
                    nc.vector.tensor_tensor(dxt[:, s], r2x1[:], r2x2[:],
                                            AOP.subtract)
                    nc.scalar.activation(c07[:, s], fa1217, ACT.Identity,
                                         bias=col(S_GA1217, j))
                    nc.scalar.activation(cab[:, s], fa517, ACT.Abs,
                                         bias=col(S_GA517, j))

                it = wp.tile([P, G * Q], F32, tag="it", name=f"it{t}")
                nc.vector.tensor_tensor(it[:], dyt[:], dxt[:], AOP.mult)
                ip = wp.tile([P, G * Q], F32, tag="ip", name=f"ip{t}")
                nc.vector.tensor_tensor(ip[:], it[:], c07[:], AOP.subtract)
                a1 = wp.tile([P, G * Q], F32, tag="a1", name=f"a1{t}")
                nc.scalar.activation(a1[:], ip[:], ACT.Abs)
                gg = wp.tile([P, G * Q], F32, tag="gg", name=f"gg{t}")
                nc.vector.tensor_tensor(gg[:], a1[:], cab[:], AOP.subtract)

                t1 = gp.tile([P, Q], F32, tag="t1", name=f"t1{t}")
                t2 = gp.tile([P, Q], F32, tag="t2", name=f"t2{t}")
                macc = gp.tile([P, Q], F32, tag="macc", name=f"macc{t}")
                nc.vector.tensor_tensor(t1[:], gg[:, 0:Q], gg[:, Q:2 * Q],
                                        AOP.min)
                nc.vector.tensor_tensor(t2[:], gg[:, 2 * Q:3 * Q],
                                        gg[:, 3 * Q:4 * Q], AOP.min)
                nc.vector.tensor_tensor(macc[:], t1[:], t2[:], AOP.min)
                nc.sync.dma_start(out_d[:, t * Q:(t + 1) * Q], macc[:])

    nc.finalize()
    return nc


def host_prep(inputs, plan):
    bboxess = np.asarray(inputs["bboxess"], dtype=np.float32)
    gt_bboxess = np.asarray(inputs["gt_bboxess"], dtype=np.float32)
    units = plan["units"]
    T = plan["T"]
    B = bboxess.shape[0]

    # per-batch feature rows and gt scalar tables
    y1 = bboxess[:, :, 0]
    x1 = bboxess[:, :, 1]
    y2 = bboxess[:, :, 2]
    x2 = bboxess[:, :, 3]
    area = ((y2 - y1) * (x2 - x1)).astype(np.float32)
    a1217 = (C1217 * area).astype(np.float32)
    a517 = (C517 * area).astype(np.float32)
    gy1 = gt_bboxess[:, :, 0]
    gx1 = gt_bboxess[:, :, 1]
    gy2 = gt_bboxess[:, :, 2]
    gx2 = gt_bboxess[:, :, 3]
    ga = ((gy2 - gy1) * (gx2 - gx1)).astype(np.float32)
    gtab = {
        S_NGY1: -gy1, S_D21Y: (gy2 - gy1), S_NGX1: -gx1, S_E21X: (gx2 - gx1),
        S_GA1217: (C1217 * ga).astype(np.float32),
        S_GA517: (C517 * ga).astype(np.float32),
    }
    feats = (y1, y2, x1, x2, a1217, a517)

    n_slots = N_CORES * P * T
    in_maps = []
    for c in range(N_CORES):
        feat = np.zeros((P, T * FQ), dtype=np.float32)
        scal = np.zeros((P, T * SCAL_W), dtype=np.float32)
        for t in range(T):
            for p in range(P):
                u = t * (N_CORES * P) + p * N_CORES + c
                if u >= len(units):
                    u = 0
                b, n0, L, gt_idx = units[u]
                base = t * FQ
                for fi, f in enumerate(feats):
                    dst = feat[p, base + fi * Q: base + fi * Q + L]
                    dst[:] = f[b, n0:n0 + L]
                    if L < Q:
                        feat[p, base + fi * Q + L: base + (fi + 1) * Q] = \
                            f[b, n0]
                sb = t * SCAL_W
                for fld, tab in gtab.items():
                    for j in range(G):
                        scal[p, sb + fld + j] = tab[b, gt_idx[j]]
        in_maps.append({"feat": feat, "scal": scal})
    return in_maps


def host_post(results, plan, inputs):
    counts = inputs["counts"]
    out_dtype = np.int64 if counts.dtype == np.int64 else np.int32
    B = counts.shape[0]
    N = inputs["bboxess"].shape[1]
    units = plan["units"]
    T = plan["T"]
    labels = np.zeros((B, N, 1), dtype=out_dtype)
    for c in range(N_CORES):
        o = results[c]["out"]   # [P, T*Q] f32 min-margin values
        fire = o <= 0.0
        for t in range(T):
            for p in range(P):
                u = t * (N_CORES * P) + p * N_CORES + c
                if u >= len(units):
                    continue
                b, n0, L, _ = units[u]
                seg = fire[p, t * Q: t * Q + L]
                np.logical_or(labels[b, n0:n0 + L, 0], seg,
                              out=labels[b, n0:n0 + L, 0],
                              casting="unsafe")
    return labels


def _axon_reset():
    import ctypes
    try:
        lib = ctypes.CDLL("/opt/axon/libaxon_pjrt.so")
        lib.axon_reset.restype = ctypes.c_int64
        lib.axon_reset()
    except Exception:
        pass


def kernel(bboxess, gt_bboxess, gt_counts, counts):
    inputs = {"bboxess": np.asarray(bboxess),
              "gt_bboxess": np.asarray(gt_bboxess),
              "gt_counts": np.asarray(gt_counts),
              "counts": np.asarray(counts)}
    plan = make_plan(inputs)
    nc = build_graph(plan)
    in_maps = host_prep(inputs, plan)
    try:
        res = run_bass_kernel_spmd(nc, in_maps, core_ids=list(range(N_CORES)))
    except Exception:
        _axon_reset()
        res = run_bass_kernel_spmd(nc, in_maps, core_ids=list(range(N_CORES)))
    return host_post(res.results, plan, inputs)


# revision 5
# speedup vs baseline: 1.0983x; 1.0983x over previous
"""Trainium2 Bass kernel for AssignClsLabel (clipped-IoU >= 0.7 proposal labeling).

Problem: bboxess [8, 65536, 4] f32, gt_bboxess [8, 64, 4] f32,
gt_counts/counts [8,1] int. Output labels [8, 65536, 1] int (0/1).

Only proposals n < count_b and gts a < gt_count_b matter (labels elsewhere
are 0), which is ~16% of the full N*A pair grid for this dataset. The
kernel therefore packs WORK UNITS = (batch b, chunk of Q proposals,
group of G gts) across 8 cores x 128 partitions x T iterations; every
partition-slot carries its own per-gt scalar columns, so different
partitions process different batches simultaneously.

Device math per pair (all f32; validated 0 label flips vs reference on
the fixed dataset):
    r2(v) = relu(d21 - relu(v - g1))       [= g2 - clip(v, g1, g2)]
    dy = r2(y1) - r2(y2)   (= clip(y2) - clip(y1));  dx likewise
    i  = dy * dx
    c  = area + ga;  fire <=> (0.41c - i)(c - i) <= 0
                     <=> |i - (12/17) c| - (5/17)|c| <= 0
Engine split: ACT does the 8 relu clips + c-terms (Identity/Abs with
per-partition bias columns, ~0.54 ns/elem packed); DVE does the 6
two-tensor ops (1.08 ns/elem). Both engines land ~balanced.
Invalid/duplicate gt slots are filled with real gts (min over
duplicates is harmless) or a far box that can never fire.
"""
import sys

import numpy as np

if "/opt/trn_rl_repo" not in sys.path:
    sys.path.insert(0, "/opt/trn_rl_repo")

import concourse.mybir as mybir
import concourse.tile as tile
from concourse import bacc
from concourse.bass_utils import run_bass_kernel_spmd

AOP = mybir.AluOpType
ACT = mybir.ActivationFunctionType
F32 = mybir.dt.float32

P = 128          # SBUF partitions
Q = 512          # proposals per work unit
G = 4            # gts per work unit
N_CORES = 8
C1217 = np.float32(12.0 / 17.0)
C517 = np.float32(5.0 / 17.0)

# scal columns per slot, per gt j in 0..G-1
S_NGY1 = 0       # -gy1
S_D21Y = G       # gy2 - gy1
S_NGX1 = 2 * G   # -gx1
S_E21X = 3 * G   # gx2 - gx1
S_GA1217 = 4 * G  # (12/17) ga
S_GA517 = 5 * G   # (5/17) ga
SCAL_W = 6 * G

FQ = 6 * Q       # feature width per slot: y1,y2,x1,x2,area1217,area517


def make_plan(inputs):
    counts = inputs["counts"]
    gt_counts = inputs["gt_counts"]
    B = counts.shape[0]
    units = []   # (b, n0, L, gt_idx tuple)
    for b in range(B):
        cnt = int(counts[b, 0])
        gcnt = int(gt_counts[b, 0])
        if cnt <= 0 or gcnt <= 0:
            continue
        nchunks = -(-cnt // Q) if cnt >= Q else 1
        ngroups = -(-gcnt // G)
        for k in range(nchunks):
            n0 = min(k * Q, max(0, cnt - Q))
            L = min(Q, cnt - n0)
            for g in range(ngroups):
                a0 = min(g * G, max(0, gcnt - G))
                gt_idx = tuple(min(a0 + j, gcnt - 1) for j in range(G))
                units.append((b, n0, L, gt_idx))
    T = -(-len(units) // (N_CORES * P))
    return {"units": units, "T": T}


def build_graph(plan):
    T = plan["T"]
    nc = bacc.Bacc()
    feat_d = nc.declare_dram_parameter("feat", [P, T * FQ], F32, isOutput=False)
    scal_d = nc.declare_dram_parameter("scal", [P, T * SCAL_W], F32,
                                       isOutput=False)
    out_d = nc.declare_dram_parameter("out", [P, T * Q], F32, isOutput=True)

    with tile.TileContext(nc) as tc:
        with (
            tc.tile_pool(name="ft", bufs=2) as fp,
            tc.tile_pool(name="gt", bufs=2) as gp,
            tc.tile_pool(name="wide", bufs=2) as wp,
        ):
            for t in range(T):
                ftile = fp.tile([P, FQ], F32, tag="feat", name=f"feat{t}")
                stile = fp.tile([P, SCAL_W], F32, tag="scal", name=f"scal{t}")
                nc.sync.dma_start(ftile[:], feat_d[:, t * FQ:(t + 1) * FQ])
                nc.sync.dma_start(stile[:], scal_d[:,
                                                   t * SCAL_W:(t + 1) * SCAL_W])
                fy1 = ftile[:, 0 * Q:1 * Q]
                fy2 = ftile[:, 1 * Q:2 * Q]
                fx1 = ftile[:, 2 * Q:3 * Q]
                fx2 = ftile[:, 3 * Q:4 * Q]
                fa1217 = ftile[:, 4 * Q:5 * Q]
                fa517 = ftile[:, 5 * Q:6 * Q]

                def col(base, j, stile=stile):
                    return stile[:, base + j:base + j + 1]

                fy12 = ftile[:, 0 * Q:2 * Q]   # (y1 || y2) contiguous
                fx12 = ftile[:, 2 * Q:4 * Q]   # (x1 || x2) contiguous
                dyt = wp.tile([P, G * Q], F32, tag="dyt", name=f"dyt{t}")
                dxt = wp.tile([P, G * Q], F32, tag="dxt", name=f"dxt{t}")
                c07 = wp.tile([P, G * Q], F32, tag="c07", name=f"c07{t}")
                cab = wp.tile([P, G * Q], F32, tag="cab", name=f"cab{t}")
                for j in range(G):
                    s = slice(j * Q, (j + 1) * Q)
                    r1y = gp.tile([P, 2 * Q], F32, tag="r1y", name=f"r1y{t}_{j}")
                    r2y = gp.tile([P, 2 * Q], F32, tag="r2y", name=f"r2y{t}_{j}")
                    r1x = gp.tile([P, 2 * Q], F32, tag="r1x", name=f"r1x{t}_{j}")
                    r2x = gp.tile([P, 2 * Q], F32, tag="r2x", name=f"r2x{t}_{j}")
                    # (y1||y2) clipped together: same bias for both halves
                    nc.scalar.activation(r1y[:], fy12, ACT.Relu,
                                         bias=col(S_NGY1, j))
                    nc.scalar.activation(r1x[:], fx12, ACT.Relu,
                                         bias=col(S_NGX1, j))
                    nc.scalar.activation(r2y[:], r1y[:], ACT.Relu,
                                         bias=col(S_D21Y, j), scale=-1.0)
                    nc.scalar.activation(r2x[:], r1x[:], ACT.Relu,
                                         bias=col(S_E21X, j), scale=-1.0)
                    # dy = r2(y1) - r2(y2) = clip(y2) - clip(y1)
                    nc.vector.tensor_tensor(dyt[:, s], r2y[:, 0:Q],
                                            r2y[:, Q:2 * Q], AOP.subtract)
                    nc.vector.tensor_tensor(dxt[:, s], r2x[:, 0:Q],
                                            r2x[:, Q:2 * Q], AOP.subtract)
                    nc.scalar.activation(c07[:, s], fa1217, ACT.Identity,
                                         bias=col(S_GA1217, j))
                    nc.scalar.activation(cab[:, s], fa517, ACT.Abs,
                                         bias=col(S_GA517, j))

                it = wp.tile([P, G * Q], F32, tag="it", name=f"it{t}")
                nc.vector.tensor_tensor(it[:], dyt[:], dxt[:], AOP.mult)
                ip = wp.tile([P, G * Q], F32, tag="ip", name=f"ip{t}")
                nc.vector.tensor_tensor(ip[:], it[:], c07[:], AOP.subtract)
                a1 = wp.tile([P, G * Q], F32, tag="a1", name=f"a1{t}")
                nc.scalar.activation(a1[:], ip[:], ACT.Abs)
                gg = wp.tile([P, G * Q], F32, tag="gg", name=f"gg{t}")
                nc.vector.tensor_tensor(gg[:], a1[:], cab[:], AOP.subtract)

                t1 = gp.tile([P, Q], F32, tag="t1", name=f"t1{t}")
                t2 = gp.tile([P, Q], F32, tag="t2", name=f"t2{t}")
                macc = gp.tile([P, Q], F32, tag="macc", name=f"macc{t}")
                nc.vector.tensor_tensor(t1[:], gg[:, 0:Q], gg[:, Q:2 * Q],
                                        AOP.min)
                nc.vector.tensor_tensor(t2[:], gg[:, 2 * Q:3 * Q],
                                        gg[:, 3 * Q:4 * Q], AOP.min)
                nc.vector.tensor_tensor(macc[:], t1[:], t2[:], AOP.min)
                nc.sync.dma_start(out_d[:, t * Q:(t + 1) * Q], macc[:])

    nc.finalize()
    return nc


def host_prep(inputs, plan):
    bboxess = np.asarray(inputs["bboxess"], dtype=np.float32)
    gt_bboxess = np.asarray(inputs["gt_bboxess"], dtype=np.float32)
    units = plan["units"]
    T = plan["T"]
    B = bboxess.shape[0]

    # per-batch feature rows and gt scalar tables
    y1 = bboxess[:, :, 0]
    x1 = bboxess[:, :, 1]
    y2 = bboxess[:, :, 2]
    x2 = bboxess[:, :, 3]
    area = ((y2 - y1) * (x2 - x1)).astype(np.float32)
    a1217 = (C1217 * area).astype(np.float32)
    a517 = (C517 * area).astype(np.float32)
    gy1 = gt_bboxess[:, :, 0]
    gx1 = gt_bboxess[:, :, 1]
    gy2 = gt_bboxess[:, :, 2]
    gx2 = gt_bboxess[:, :, 3]
    ga = ((gy2 - gy1) * (gx2 - gx1)).astype(np.float32)
    gtab = {
        S_NGY1: -gy1, S_D21Y: (gy2 - gy1), S_NGX1: -gx1, S_E21X: (gx2 - gx1),
        S_GA1217: (C1217 * ga).astype(np.float32),
        S_GA517: (C517 * ga).astype(np.float32),
    }
    feats = (y1, y2, x1, x2, a1217, a517)

    n_slots = N_CORES * P * T
    in_maps = []
    for c in range(N_CORES):
        feat = np.zeros((P, T * FQ), dtype=np.float32)
        scal = np.zeros((P, T * SCAL_W), dtype=np.float32)
        for t in range(T):
            for p in range(P):
                u = t * (N_CORES * P) + p * N_CORES + c
                if u >= len(units):
                    u = 0
                b, n0, L, gt_idx = units[u]
                base = t * FQ
                for fi, f in enumerate(feats):
                    dst = feat[p, base + fi * Q: base + fi * Q + L]
                    dst[:] = f[b, n0:n0 + L]
                    if L < Q:
                        feat[p, base + fi * Q + L: base + (fi + 1) * Q] = \
                            f[b, n0]
                sb = t * SCAL_W
                for fld, tab in gtab.items():
                    for j in range(G):
                        scal[p, sb + fld + j] = tab[b, gt_idx[j]]
        in_maps.append({"feat": feat, "scal": scal})
    return in_maps


def host_post(results, plan, inputs):
    counts = inputs["counts"]
    out_dtype = np.int64 if counts.dtype == np.int64 else np.int32
    B = counts.shape[0]
    N = inputs["bboxess"].shape[1]
    units = plan["units"]
    T = plan["T"]
    labels = np.zeros((B, N, 1), dtype=out_dtype)
    for c in range(N_CORES):
        o = results[c]["out"]   # [P, T*Q] f32 min-margin values
        fire = o <= 0.0
        for t in range(T):
            for p in range(P):
                u = t * (N_CORES * P) + p * N_CORES + c
                if u >= len(units):
                    continue
                b, n0, L, _ = units[u]
                seg = fire[p, t * Q: t * Q + L]
                np.logical_or(labels[b, n0:n0 + L, 0], seg,
                              out=labels[b, n0:n0 + L, 0],
                              casting="unsafe")
    return labels


def _axon_reset():
    import ctypes
    try:
        lib = ctypes.CDLL("/opt/axon/libaxon_pjrt.so")
        lib.axon_reset.restype = ctypes.c_int64
        lib.axon_reset()
    except Exception:
        pass


def kernel(bboxess, gt_bboxess, gt_counts, counts):
    inputs = {"bboxess": np.asarray(bboxess),
              "gt_bboxess": np.asarray(gt_bboxess),
              "gt_counts": np.asarray(gt_counts),
              "counts": np.asarray(counts)}
    plan = make_plan(inputs)
    nc = build_graph(plan)
    in_maps = host_prep(inputs, plan)
    try:
        res = run_bass_kernel_spmd(nc, in_maps, core_ids=list(range(N_CORES)))
    except Exception:
        _axon_reset()
        res = run_bass_kernel_spmd(nc, in_maps, core_ids=list(range(N_CORES)))
    return host_post(res.results, plan, inputs)


# revision 7
# speedup vs baseline: 1.5097x; 1.3746x over previous
"""Trainium2 Bass kernel for AssignClsLabel (clipped-IoU >= 0.7 proposal labeling).

Problem: bboxess [8, 65536, 4] f32, gt_bboxess [8, 64, 4] f32,
gt_counts/counts [8,1] int. Output labels [8, 65536, 1] int (0/1).

Only proposals n < count_b and gts a < gt_count_b matter (~16% of the
full N*A grid here), so work is packed as UNITS = (batch b, chunk of
Q=704 proposals, group of G=4 gts) spread over 8 cores x 128
partitions x T iterations; every partition-slot carries its own
per-gt scalar columns (tensor_scalar / activation bias operands are
per-partition), so different partitions process different batches in
the same instruction.

Device math per pair (all f32; 0 label flips vs reference on the
fixed dataset):
    clip(v) into [g1,g2]; dy = clip(y2)-clip(y1); dx likewise
    i = dy*dx;  c = area + ga
    fire <=> (i - (12/17)c)^2 <= ((5/17)c)^2 <=> |i-(12/17)c|-(5/17)|c| <= 0
Engine split (rates measured on HW): DVE tensor_scalar 2-op ~0.6ns/elem
does clips for gts 0,1 (max,min fused); ACT (~1.0ns/elem) does relu-pair
clips for gts 2,3 plus the c-terms (Identity/Abs with fused input scale
and per-partition ga bias) and |i - c07|; DVE tensor_tensor (1.1ns/elem)
does dy/dx/i/ip/gg and the 4-gt min tree.
"""
import sys

import numpy as np

if "/opt/trn_rl_repo" not in sys.path:
    sys.path.insert(0, "/opt/trn_rl_repo")

import concourse.mybir as mybir
import concourse.tile as tile
from concourse import bacc
from concourse.bass_utils import run_bass_kernel_spmd

AOP = mybir.AluOpType
ACT = mybir.ActivationFunctionType
F32 = mybir.dt.float32

P = 128          # SBUF partitions
Q = 704          # proposals per work unit
G = 4            # gts per work unit
TS_GTS = (0, 1)  # gts clipped on DVE tensor_scalar
N_CORES = 8
F1217 = float(np.float32(12.0 / 17.0))
F517 = float(np.float32(5.0 / 17.0))

# scal columns per slot, per gt j in 0..G-1
S_GY1 = 0 * G
S_GY2 = 1 * G
S_GX1 = 2 * G
S_GX2 = 3 * G
S_NGY1 = 4 * G   # -gy1
S_D21Y = 5 * G   # gy2 - gy1
S_NGX1 = 6 * G   # -gx1
S_E21X = 7 * G   # gx2 - gx1
S_GA1217 = 8 * G  # (12/17) ga
S_GA517 = 9 * G   # (5/17) ga
SCAL_W = 10 * G

FQ = 5 * Q       # feature width per slot: y1,y2,x1,x2,area


def make_plan(inputs):
    counts = inputs["counts"]
    gt_counts = inputs["gt_counts"]
    B = counts.shape[0]
    units = []   # (b, n0, L, gt_idx tuple)
    for b in range(B):
        cnt = int(counts[b, 0])
        gcnt = int(gt_counts[b, 0])
        if cnt <= 0 or gcnt <= 0:
            continue
        nchunks = -(-cnt // Q) if cnt >= Q else 1
        ngroups = -(-gcnt // G)
        for k in range(nchunks):
            n0 = min(k * Q, max(0, cnt - Q))
            L = min(Q, cnt - n0)
            for g in range(ngroups):
                a0 = min(g * G, max(0, gcnt - G))
                gt_idx = tuple(min(a0 + j, gcnt - 1) for j in range(G))
                units.append((b, n0, L, gt_idx))
    T = -(-len(units) // (N_CORES * P))
    return {"units": units, "T": T}


def build_graph(plan):
    T = plan["T"]
    nc = bacc.Bacc()
    feat_d = nc.declare_dram_parameter("feat", [P, T * FQ], F32, isOutput=False)
    scal_d = nc.declare_dram_parameter("scal", [P, T * SCAL_W], F32,
                                       isOutput=False)
    out_d = nc.declare_dram_parameter("out", [P, T * Q], F32, isOutput=True)

    with tile.TileContext(nc) as tc:
        with (
            tc.tile_pool(name="ft", bufs=2) as fp,
            tc.tile_pool(name="gt", bufs=2) as gp,
            tc.tile_pool(name="wide", bufs=1) as wp,
            tc.tile_pool(name="cpool", bufs=2) as cp,
            tc.tile_pool(name="big", bufs=2) as bp,
            tc.tile_pool(name="sm", bufs=1) as sp,
        ):
            for t in range(T):
                ftile = fp.tile([P, FQ], F32, tag="feat", name=f"feat{t}")
                stile = fp.tile([P, SCAL_W], F32, tag="scal", name=f"scal{t}")
                nc.sync.dma_start(ftile[:], feat_d[:, t * FQ:(t + 1) * FQ])
                nc.sync.dma_start(stile[:], scal_d[:,
                                                   t * SCAL_W:(t + 1) * SCAL_W])
                fy12 = ftile[:, 0 * Q:2 * Q]   # (y1 || y2)
                fx12 = ftile[:, 2 * Q:4 * Q]   # (x1 || x2)
                farea = ftile[:, 4 * Q:5 * Q]

                def col(base, j, stile=stile):
                    return stile[:, base + j:base + j + 1]

                dyt = wp.tile([P, G * Q], F32, tag="dyt", name=f"dyt{t}")
                dxt = wp.tile([P, G * Q], F32, tag="dxt", name=f"dxt{t}")
                c07 = cp.tile([P, G * Q], F32, tag="c07", name=f"c07{t}")
                cab = cp.tile([P, G * Q], F32, tag="cab", name=f"cab{t}")
                for j in range(G):
                    s = slice(j * Q, (j + 1) * Q)
                    if j in TS_GTS:
                        # DVE: clip both coords in one fused (max,min) ts
                        yy = gp.tile([P, 2 * Q], F32, tag="cly",
                                     name=f"yy{t}_{j}")
                        xx = gp.tile([P, 2 * Q], F32, tag="clx",
                                     name=f"xx{t}_{j}")
                        nc.vector.tensor_scalar(
                            yy[:], fy12, col(S_GY1, j), col(S_GY2, j),
                            AOP.max, AOP.min)
                        nc.vector.tensor_scalar(
                            xx[:], fx12, col(S_GX1, j), col(S_GX2, j),
                            AOP.max, AOP.min)
                        # dy = clip(y2) - clip(y1)
                        nc.vector.tensor_tensor(dyt[:, s], yy[:, Q:2 * Q],
                                                yy[:, 0:Q], AOP.subtract)
                        nc.vector.tensor_tensor(dxt[:, s], xx[:, Q:2 * Q],
                                                xx[:, 0:Q], AOP.subtract)
                    else:
                        # ACT: r2(v) = relu(d21 - relu(v - g1)) = g2 - clip(v)
                        r1y = gp.tile([P, 2 * Q], F32, tag="r1",
                                      name=f"r1y{t}_{j}")
                        r1x = gp.tile([P, 2 * Q], F32, tag="r1",
                                      name=f"r1x{t}_{j}")
                        r2y = gp.tile([P, 2 * Q], F32, tag="cly",
                                      name=f"r2y{t}_{j}")
                        r2x = gp.tile([P, 2 * Q], F32, tag="clx",
                                      name=f"r2x{t}_{j}")
                        nc.scalar.activation(r1y[:], fy12, ACT.Relu,
                                             bias=col(S_NGY1, j))
                        nc.scalar.activation(r1x[:], fx12, ACT.Relu,
                                             bias=col(S_NGX1, j))
                        nc.scalar.activation(r2y[:], r1y[:], ACT.Relu,
                                             bias=col(S_D21Y, j), scale=-1.0)
                        nc.scalar.activation(r2x[:], r1x[:], ACT.Relu,
                                             bias=col(S_E21X, j), scale=-1.0)
                        # dy = r2(y1) - r2(y2) = clip(y2) - clip(y1)
                        nc.vector.tensor_tensor(dyt[:, s], r2y[:, 0:Q],
                                                r2y[:, Q:2 * Q], AOP.subtract)
                        nc.vector.tensor_tensor(dxt[:, s], r2x[:, 0:Q],
                                                r2x[:, Q:2 * Q], AOP.subtract)
                    # c-terms: scale folded into ACT input transform
                    nc.scalar.activation(c07[:, s], farea, ACT.Identity,
                                         bias=col(S_GA1217, j), scale=F1217)
                    nc.scalar.activation(cab[:, s], farea, ACT.Abs,
                                         bias=col(S_GA517, j), scale=F517)

                it = bp.tile([P, G * Q], F32, tag="big", name=f"it{t}")
                nc.vector.tensor_tensor(it[:], dyt[:], dxt[:], AOP.mult)
                ip = bp.tile([P, G * Q], F32, tag="big", name=f"ip{t}")
                nc.vector.tensor_tensor(ip[:], it[:], c07[:], AOP.subtract)
                a1 = bp.tile([P, G * Q], F32, tag="big", name=f"a1{t}")
                nc.scalar.activation(a1[:], ip[:], ACT.Abs)
                gg = bp.tile([P, G * Q], F32, tag="big", name=f"gg{t}")
                nc.vector.tensor_tensor(gg[:], a1[:], cab[:], AOP.subtract)

                t1 = sp.tile([P, Q], F32, tag="t1", name=f"t1{t}")
                t2 = sp.tile([P, Q], F32, tag="t2", name=f"t2{t}")
                macc = sp.tile([P, Q], F32, tag="macc", name=f"macc{t}")
                nc.vector.tensor_tensor(t1[:], gg[:, 0:Q], gg[:, Q:2 * Q],
                                        AOP.min)
                nc.vector.tensor_tensor(t2[:], gg[:, 2 * Q:3 * Q],
                                        gg[:, 3 * Q:4 * Q], AOP.min)
                nc.vector.tensor_tensor(macc[:], t1[:], t2[:], AOP.min)
                nc.sync.dma_start(out_d[:, t * Q:(t + 1) * Q], macc[:])

    nc.finalize()
    return nc


def host_prep(inputs, plan):
    bboxess = np.asarray(inputs["bboxess"], dtype=np.float32)
    gt_bboxess = np.asarray(inputs["gt_bboxess"], dtype=np.float32)
    units = plan["units"]
    T = plan["T"]

    y1 = bboxess[:, :, 0]
    x1 = bboxess[:, :, 1]
    y2 = bboxess[:, :, 2]
    x2 = bboxess[:, :, 3]
    area = ((y2 - y1) * (x2 - x1)).astype(np.float32)
    gy1 = gt_bboxess[:, :, 0]
    gx1 = gt_bboxess[:, :, 1]
    gy2 = gt_bboxess[:, :, 2]
    gx2 = gt_bboxess[:, :, 3]
    ga = ((gy2 - gy1) * (gx2 - gx1)).astype(np.float32)
    gtab = {
        S_GY1: gy1, S_GY2: gy2, S_GX1: gx1, S_GX2: gx2,
        S_NGY1: -gy1, S_D21Y: (gy2 - gy1), S_NGX1: -gx1, S_E21X: (gx2 - gx1),
        S_GA1217: (np.float32(F1217) * ga).astype(np.float32),
        S_GA517: (np.float32(F517) * ga).astype(np.float32),
    }
    feats = (y1, y2, x1, x2, area)

    in_maps = []
    for c in range(N_CORES):
        feat = np.zeros((P, T * FQ), dtype=np.float32)
        scal = np.zeros((P, T * SCAL_W), dtype=np.float32)
        for t in range(T):
            for p in range(P):
                u = t * (N_CORES * P) + p * N_CORES + c
                if u >= len(units):
                    u = 0
                b, n0, L, gt_idx = units[u]
                base = t * FQ
                for fi, f in enumerate(feats):
                    dst = feat[p, base + fi * Q: base + fi * Q + L]
                    dst[:] = f[b, n0:n0 + L]
                    if L < Q:
                        feat[p, base + fi * Q + L: base + (fi + 1) * Q] = \
                            f[b, n0]
                sb = t * SCAL_W
                for fld, tab in gtab.items():
                    for j in range(G):
                        scal[p, sb + fld + j] = tab[b, gt_idx[j]]
        in_maps.append({"feat": feat, "scal": scal})
    return in_maps


def host_post(results, plan, inputs):
    counts = inputs["counts"]
    out_dtype = np.int64 if counts.dtype == np.int64 else np.int32
    B = counts.shape[0]
    N = inputs["bboxess"].shape[1]
    units = plan["units"]
    T = plan["T"]
    labels = np.zeros((B, N, 1), dtype=out_dtype)
    for c in range(N_CORES):
        o = results[c]["out"]   # [P, T*Q] f32 min-margin values
        fire = o <= 0.0
        for t in range(T):
            for p in range(P):
                u = t * (N_CORES * P) + p * N_CORES + c
                if u >= len(units):
                    continue
                b, n0, L, _ = units[u]
                seg = fire[p, t * Q: t * Q + L]
                np.logical_or(labels[b, n0:n0 + L, 0], seg,
                              out=labels[b, n0:n0 + L, 0],
                              casting="unsafe")
    return labels


def _axon_reset():
    import ctypes
    try:
        lib = ctypes.CDLL("/opt/axon/libaxon_pjrt.so")
        lib.axon_reset.restype = ctypes.c_int64
        lib.axon_reset()
    except Exception:
        pass


def kernel(bboxess, gt_bboxess, gt_counts, counts):
    inputs = {"bboxess": np.asarray(bboxess),
              "gt_bboxess": np.asarray(gt_bboxess),
              "gt_counts": np.asarray(gt_counts),
              "counts": np.asarray(counts)}
    plan = make_plan(inputs)
    nc = build_graph(plan)
    in_maps = host_prep(inputs, plan)
    try:
        res = run_bass_kernel_spmd(nc, in_maps, core_ids=list(range(N_CORES)))
    except Exception:
        _axon_reset()
        res = run_bass_kernel_spmd(nc, in_maps, core_ids=list(range(N_CORES)))
    return host_post(res.results, plan, inputs)


# revision 10
# speedup vs baseline: 1.5291x; 1.0128x over previous
"""Trainium2 Bass kernel for AssignClsLabel (clipped-IoU >= 0.7 proposal labeling).

Problem: bboxess [8, 65536, 4] f32, gt_bboxess [8, 64, 4] f32,
gt_counts/counts [8,1] int. Output labels [8, 65536, 1] int (0/1).

Only proposals n < count_b and gts a < gt_count_b matter (~16% of the
full N*A grid here), so work is packed as UNITS = (batch b, chunk of
Q=704 proposals, group of G=4 gts) spread over 8 cores x 128
partitions x T iterations; every partition-slot carries its own
per-gt scalar columns (tensor_scalar / activation bias operands are
per-partition), so different partitions process different batches in
the same instruction.

Device math per pair (all f32; 0 label flips vs reference on the
fixed dataset):
    clip(v) into [g1,g2]; dy = clip(y2)-clip(y1); dx likewise
    i = dy*dx;  c = area + ga
    fire <=> (i - (12/17)c)^2 <= ((5/17)c)^2 <=> |i-(12/17)c|-(5/17)|c| <= 0
Engine split (rates measured on HW): DVE tensor_scalar 2-op ~0.6ns/elem
does clips for gts 0,1 (max,min fused); ACT (~1.0ns/elem) does relu-pair
clips for gts 2,3 plus the c-terms (Identity/Abs with fused input scale
and per-partition ga bias) and |i - c07|; DVE tensor_tensor (1.1ns/elem)
does dy/dx/i/ip/gg and the 4-gt min tree.
"""
import sys

import numpy as np

if "/opt/trn_rl_repo" not in sys.path:
    sys.path.insert(0, "/opt/trn_rl_repo")

import concourse.mybir as mybir
import concourse.tile as tile
from concourse import bacc
from concourse.bass_utils import run_bass_kernel_spmd

AOP = mybir.AluOpType
ACT = mybir.ActivationFunctionType
F32 = mybir.dt.float32

P = 128          # SBUF partitions
Q = 704          # proposals per work unit
G = 4            # gts per work unit
# clip routing: (gt, axis) pairs handled by DVE tensor_scalar; rest on ACT
TS_AXES = {(0, 0), (0, 1), (1, 0)}
N_CORES = 8
F1217 = float(np.float32(12.0 / 17.0))
F517 = float(np.float32(5.0 / 17.0))

# scal columns per slot, per gt j in 0..G-1
S_GY1 = 0 * G
S_GY2 = 1 * G
S_GX1 = 2 * G
S_GX2 = 3 * G
S_NGY1 = 4 * G   # -gy1
S_D21Y = 5 * G   # gy2 - gy1
S_NGX1 = 6 * G   # -gx1
S_E21X = 7 * G   # gx2 - gx1
S_GA1217 = 8 * G  # (12/17) ga
S_GA517 = 9 * G   # (5/17) ga
SCAL_W = 10 * G

FQ = 6 * Q       # feature width per slot: y1,y2,x1,x2,area,area1217


def make_plan(inputs):
    counts = inputs["counts"]
    gt_counts = inputs["gt_counts"]
    B = counts.shape[0]
    units = []   # (b, n0, L, gt_idx tuple)
    for b in range(B):
        cnt = int(counts[b, 0])
        gcnt = int(gt_counts[b, 0])
        if cnt <= 0 or gcnt <= 0:
            continue
        nchunks = -(-cnt // Q) if cnt >= Q else 1
        ngroups = -(-gcnt // G)
        for k in range(nchunks):
            n0 = min(k * Q, max(0, cnt - Q))
            L = min(Q, cnt - n0)
            for g in range(ngroups):
                a0 = min(g * G, max(0, gcnt - G))
                gt_idx = tuple(min(a0 + j, gcnt - 1) for j in range(G))
                units.append((b, n0, L, gt_idx))
    T = -(-len(units) // (N_CORES * P))
    return {"units": units, "T": T}


def build_graph(plan):
    T = plan["T"]
    nc = bacc.Bacc()
    feat_d = nc.declare_dram_parameter("feat", [P, T * FQ], F32, isOutput=False)
    scal_d = nc.declare_dram_parameter("scal", [P, T * SCAL_W], F32,
                                       isOutput=False)
    out_d = nc.declare_dram_parameter("out", [P, T * Q], F32, isOutput=True)

    with tile.TileContext(nc) as tc:
        with (
            tc.tile_pool(name="ft", bufs=2) as fp,
            tc.tile_pool(name="gt", bufs=2) as gp,
            tc.tile_pool(name="wide", bufs=1) as wp,
            tc.tile_pool(name="cpool", bufs=2) as cp,
            tc.tile_pool(name="big", bufs=4) as bp,
            tc.tile_pool(name="ggp", bufs=4) as ggp,
            tc.tile_pool(name="sm", bufs=1) as sp,
        ):
            for t in range(T):
                ftile = fp.tile([P, FQ], F32, tag="feat", name=f"feat{t}")
                stile = fp.tile([P, SCAL_W], F32, tag="scal", name=f"scal{t}")
                nc.sync.dma_start(stile[:], scal_d[:,
                                                   t * SCAL_W:(t + 1) * SCAL_W])
                # coords first so clips can start before areas land
                nc.sync.dma_start(ftile[:, 0:4 * Q],
                                  feat_d[:, t * FQ:t * FQ + 4 * Q])
                nc.sync.dma_start(ftile[:, 4 * Q:6 * Q],
                                  feat_d[:, t * FQ + 4 * Q:(t + 1) * FQ])
                fy12 = ftile[:, 0 * Q:2 * Q]   # (y1 || y2)
                fx12 = ftile[:, 2 * Q:4 * Q]   # (x1 || x2)
                farea = ftile[:, 4 * Q:5 * Q]
                farea12 = ftile[:, 5 * Q:6 * Q]

                def col(base, j, stile=stile):
                    return stile[:, base + j:base + j + 1]

                # clips: (gt, axis) routed to DVE ts or ACT relu-pair.
                # each produces cl[j][axis] = [P, 2Q] tile; sign tells
                # whether halves are clip(v) (ts) or g2-clip(v) (ACT).
                cl = {}
                for j in range(G):
                    for ax in (0, 1):
                        fin = fy12 if ax == 0 else fx12
                        tag = ("cly", "clx")[ax]
                        if (j, ax) in TS_AXES:
                            cc = gp.tile([P, 2 * Q], F32, tag=tag,
                                         name=f"ts{t}_{j}_{ax}")
                            lo = col((S_GY1, S_GX1)[ax], j)
                            hi = col((S_GY2, S_GX2)[ax], j)
                            nc.vector.tensor_scalar(cc[:], fin, lo, hi,
                                                    AOP.max, AOP.min)
                            cl[(j, ax)] = (cc, True)
                        else:
                            r1 = gp.tile([P, 2 * Q], F32, tag="r1",
                                         name=f"r1_{t}_{j}_{ax}")
                            cc = gp.tile([P, 2 * Q], F32, tag=tag,
                                         name=f"r2_{t}_{j}_{ax}")
                            b1 = col((S_NGY1, S_NGX1)[ax], j)
                            b2 = col((S_D21Y, S_E21X)[ax], j)
                            nc.scalar.activation(r1[:], fin, ACT.Relu, bias=b1)
                            nc.scalar.activation(cc[:], r1[:], ACT.Relu,
                                                 bias=b2, scale=-1.0)
                            cl[(j, ax)] = (cc, False)
                    # cab only needs areas + scal; emit early for overlap
                    if j == 0:
                        cab = cp.tile([P, G * Q], F32, tag="cab",
                                      name=f"cab{t}")
                    nc.scalar.activation(cab[:, j * Q:(j + 1) * Q], farea,
                                         ACT.Abs, bias=col(S_GA517, j),
                                         scale=F517)

                # per-gt pipelined tail
                ggs = []
                for j in range(G):
                    dy = gp.tile([P, Q], F32, tag="dy", name=f"dy{t}_{j}")
                    dx = gp.tile([P, Q], F32, tag="dx", name=f"dx{t}_{j}")
                    for ax, d in ((0, dy), (1, dx)):
                        cc, direct = cl[(j, ax)]
                        if direct:   # dy = clip(v2) - clip(v1)
                            nc.vector.tensor_tensor(d[:], cc[:, Q:2 * Q],
                                                    cc[:, 0:Q], AOP.subtract)
                        else:        # dy = r2(v1) - r2(v2)
                            nc.vector.tensor_tensor(d[:], cc[:, 0:Q],
                                                    cc[:, Q:2 * Q],
                                                    AOP.subtract)
                    it = bp.tile([P, Q], F32, tag="big", name=f"it{t}_{j}")
                    nc.vector.tensor_tensor(it[:], dy[:], dx[:], AOP.mult)
                    # ip = (i - (12/17)ga) - (12/17)area
                    ip = bp.tile([P, Q], F32, tag="big", name=f"ip{t}_{j}")
                    nc.vector.scalar_tensor_tensor(
                        ip[:], it[:], col(S_GA1217, j), farea12,
                        AOP.subtract, AOP.subtract)
                    a1 = bp.tile([P, Q], F32, tag="big", name=f"a1{t}_{j}")
                    nc.scalar.activation(a1[:], ip[:], ACT.Abs)
                    gg = ggp.tile([P, Q], F32, tag="gg", name=f"gg{t}_{j}")
                    nc.vector.tensor_tensor(gg[:], a1[:],
                                            cab[:, j * Q:(j + 1) * Q],
                                            AOP.subtract)
                    ggs.append(gg)

                t1 = sp.tile([P, Q], F32, tag="t1", name=f"t1{t}")
                t2 = sp.tile([P, Q], F32, tag="t2", name=f"t2{t}")
                macc = sp.tile([P, Q], F32, tag="macc", name=f"macc{t}")
                nc.vector.tensor_tensor(t1[:], ggs[0][:], ggs[1][:], AOP.min)
                nc.vector.tensor_tensor(t2[:], ggs[2][:], ggs[3][:], AOP.min)
                nc.vector.tensor_tensor(macc[:], t1[:], t2[:], AOP.min)
                nc.sync.dma_start(out_d[:, t * Q:(t + 1) * Q], macc[:])

    nc.finalize()
    return nc


def host_prep(inputs, plan):
    bboxess = np.asarray(inputs["bboxess"], dtype=np.float32)
    gt_bboxess = np.asarray(inputs["gt_bboxess"], dtype=np.float32)
    units = plan["units"]
    T = plan["T"]

    y1 = bboxess[:, :, 0]
    x1 = bboxess[:, :, 1]
    y2 = bboxess[:, :, 2]
    x2 = bboxess[:, :, 3]
    area = ((y2 - y1) * (x2 - x1)).astype(np.float32)
    gy1 = gt_bboxess[:, :, 0]
    gx1 = gt_bboxess[:, :, 1]
    gy2 = gt_bboxess[:, :, 2]
    gx2 = gt_bboxess[:, :, 3]
    ga = ((gy2 - gy1) * (gx2 - gx1)).astype(np.float32)
    gtab = {
        S_GY1: gy1, S_GY2: gy2, S_GX1: gx1, S_GX2: gx2,
        S_NGY1: -gy1, S_D21Y: (gy2 - gy1), S_NGX1: -gx1, S_E21X: (gx2 - gx1),
        S_GA1217: (np.float32(F1217) * ga).astype(np.float32),
        S_GA517: (np.float32(F517) * ga).astype(np.float32),
    }
    area1217 = (np.float32(F1217) * area).astype(np.float32)
    feats = (y1, y2, x1, x2, area, area1217)

    in_maps = []
    for c in range(N_CORES):
        feat = np.zeros((P, T * FQ), dtype=np.float32)
        scal = np.zeros((P, T * SCAL_W), dtype=np.float32)
        for t in range(T):
            for p in range(P):
                u = t * (N_CORES * P) + p * N_CORES + c
                if u >= len(units):
                    u = 0
                b, n0, L, gt_idx = units[u]
                base = t * FQ
                for fi, f in enumerate(feats):
                    dst = feat[p, base + fi * Q: base + fi * Q + L]
                    dst[:] = f[b, n0:n0 + L]
                    if L < Q:
                        feat[p, base + fi * Q + L: base + (fi + 1) * Q] = \
                            f[b, n0]
                sb = t * SCAL_W
                for fld, tab in gtab.items():
                    for j in range(G):
                        scal[p, sb + fld + j] = tab[b, gt_idx[j]]
        in_maps.append({"feat": feat, "scal": scal})
    return in_maps


def host_post(results, plan, inputs):
    counts = inputs["counts"]
    out_dtype = np.int64 if counts.dtype == np.int64 else np.int32
    B = counts.shape[0]
    N = inputs["bboxess"].shape[1]
    units = plan["units"]
    T = plan["T"]
    labels = np.zeros((B, N, 1), dtype=out_dtype)
    for c in range(N_CORES):
        o = results[c]["out"]   # [P, T*Q] f32 min-margin values
        fire = o <= 0.0
        for t in range(T):
            for p in range(P):
                u = t * (N_CORES * P) + p * N_CORES + c
                if u >= len(units):
                    continue
                b, n0, L, _ = units[u]
                seg = fire[p, t * Q: t * Q + L]
                np.logical_or(labels[b, n0:n0 + L, 0], seg,
                              out=labels[b, n0:n0 + L, 0],
                              casting="unsafe")
    return labels


def _axon_reset():
    import ctypes
    try:
        lib = ctypes.CDLL("/opt/axon/libaxon_pjrt.so")
        lib.axon_reset.restype = ctypes.c_int64
        lib.axon_reset()
    except Exception:
        pass


def kernel(bboxess, gt_bboxess, gt_counts, counts):
    inputs = {"bboxess": np.asarray(bboxess),
              "gt_bboxess": np.asarray(gt_bboxess),
              "gt_counts": np.asarray(gt_counts),
              "counts": np.asarray(counts)}
    plan = make_plan(inputs)
    nc = build_graph(plan)
    in_maps = host_prep(inputs, plan)
    try:
        res = run_bass_kernel_spmd(nc, in_maps, core_ids=list(range(N_CORES)))
    except Exception:
        _axon_reset()
        res = run_bass_kernel_spmd(nc, in_maps, core_ids=list(range(N_CORES)))
    return host_post(res.results, plan, inputs)
